# Initial kernel scaffold
#
"""Trainium2 Bass kernel for nn_CPUVisionModel (8-block ViT + merger MLP).

Sharding: sequence-parallel residual stream (256 tokens/core, dim-major
(hidden, token) layout) + head-sharded attention (2 heads/core) +
intermediate-sharded MLP (640/core).  Collectives per layer:
AllGather(LN1 out, bf16), AllToAll(attn out, bf16), AllGather(LN2 out,
bf16), ReduceScatter(fc2 partial, bf16).  Merger: AllGather(y, fp32) +
ReduceScatter(out, bf16).

qkv/fc1/fc2/attention/proj matmuls run in bf16 (fp32 PSUM accumulation);
merger matmuls in fp32r; residual stream, layernorms and reductions in
fp32.
"""
import os
import sys

sys.path.insert(0, "/opt/trn_rl_repo")

import ml_dtypes
import numpy as np

import concourse.bass as bass
import concourse.mybir as mybir
import concourse.tile as tile
from concourse import bacc
from concourse.bass_utils import run_bass_kernel_spmd
from concourse.masks import make_identity

dt = mybir.dt
AF = mybir.ActivationFunctionType
OP = mybir.AluOpType
BF = ml_dtypes.bfloat16

# model dims
S, H, NH, HD, I = 2048, 1280, 16, 80, 5120
MH, OUT = 5120, 3584
DEPTH = int(os.environ.get("KERNEL_DEPTH", "8"))
SCALE = HD ** -0.5
EPS = 1e-6

NCORE = 8
SR = S // NCORE          # 256 tokens per core
HPC = NH // NCORE        # 2 heads per core
DR = HPC * HD            # 160 attn dims per core
IR = I // NCORE          # 640 intermediate per core
GR = SR // 4             # 64 merger rows per core
HT = H // 128            # 10 hidden k-tiles
IT = IR // 128           # 5 intermediate tiles per core
ST = S // 128            # 16 token tiles
OT = OUT // 128          # 28 output tiles
MT = MH // 128           # 40 merger-hidden tiles
RG = [list(range(NCORE))]

_cached = {}


def _layernorm(nc, pools, src, wln, bln, dst, onesr, name):
    """LayerNorm over hidden axis of dim-major src (128, HT*SR) -> dst."""
    sb, ps, tiny, bc = pools["sbuf"], pools["psum"], pools["tiny"], pools["bcast"]
    xr = sb.tile([128, HT * SR], dt.float32r, tag="ln_xr", bufs=1,
                 name=f"xr_{name}")
    xsq = sb.tile([128, HT * SR], dt.float32r, tag="ln_xsq", bufs=2,
                  name=f"xsq_{name}")
    nc.scalar.activation(xr[:], src, AF.Copy)
    nc.scalar.activation(xsq[:], src, AF.Square)
    ssum = ps.tile([1, SR], dt.float32, tag="stat", bufs=2, name=f"ss_{name}")
    qsum = ps.tile([1, SR], dt.float32, tag="stat", bufs=2, name=f"qs_{name}")
    for t in range(HT):
        nc.tensor.matmul(ssum[:], onesr[:], xr[:, t * SR:(t + 1) * SR],
                         start=(t == 0), stop=(t == HT - 1))
    for t in range(HT):
        nc.tensor.matmul(qsum[:], onesr[:], xsq[:, t * SR:(t + 1) * SR],
                         start=(t == 0), stop=(t == HT - 1))
    mean = tiny.tile([1, SR], dt.float32, tag="t_mean", bufs=2, name=f"mn_{name}")
    var = tiny.tile([1, SR], dt.float32, tag="t_var", bufs=2, name=f"vr_{name}")
    std = tiny.tile([1, SR], dt.float32, tag="t_std", bufs=2, name=f"sd_{name}")
    rstd = tiny.tile([1, SR], dt.float32, tag="t_rstd", bufs=2, name=f"rs_{name}")
    msq = tiny.tile([1, SR], dt.float32, tag="t_msq", bufs=2, name=f"mq_{name}")
    nc.vector.tensor_scalar_mul(mean[:], ssum[:], 1.0 / H)
    nc.vector.tensor_scalar_mul(var[:], qsum[:], 1.0 / H)
    nc.vector.tensor_tensor(msq[:], mean[:], mean[:], OP.mult)
    nc.vector.tensor_tensor(var[:], var[:], msq[:], OP.subtract)
    nc.vector.tensor_scalar_add(var[:], var[:], EPS)
    nc.scalar.activation(std[:], var[:], AF.Sqrt)
    nc.vector.reciprocal(rstd[:], std[:])
    meanb = bc.tile([128, SR], dt.float32, tag="b_mean", bufs=2, name=f"mb_{name}")
    rstdb = bc.tile([128, SR], dt.float32, tag="b_rstd", bufs=2, name=f"rb_{name}")
    nc.gpsimd.partition_broadcast(meanb[:], mean[:])
    nc.gpsimd.partition_broadcast(rstdb[:], rstd[:])
    tmp = sb.tile([128, HT * SR], dt.float32, tag="ln_xsq", bufs=2,
                  name=f"lt_{name}")
    srcv = src.rearrange("p (t s) -> p t s", t=HT)
    tmpv = tmp[:].rearrange("p (t s) -> p t s", t=HT)
    dstv = dst.rearrange("p (t s) -> p t s", t=HT)
    mb = meanb[:].unsqueeze(1).broadcast_to((128, HT, SR))
    rb = rstdb[:].unsqueeze(1).broadcast_to((128, HT, SR))
    wv = wln.unsqueeze(2).broadcast_to((128, HT, SR))
    bv = bln.unsqueeze(2).broadcast_to((128, HT, SR))
    nc.vector.tensor_tensor(tmpv, srcv, mb, OP.subtract)
    nc.vector.tensor_tensor(tmpv, tmpv, rb, OP.mult)
    nc.vector.tensor_tensor(tmpv, tmpv, wv, OP.mult)
    nc.vector.tensor_tensor(dstv, tmpv, bv, OP.add)


def build_nc(depth=DEPTH, num_cores=NCORE):
    nc = bacc.Bacc("TRN2", target_bir_lowering=False, debug=False,
                   enable_asserts=True, num_devices=num_cores)

    inp = {}

    def din(name, shape, dd=dt.float32):
        inp[name] = nc.dram_tensor(name, list(shape), dd,
                                   kind="ExternalInput").ap()
        return inp[name]

    din("x0", (H, SR))
    din("cosT", (HD, S), dt.bfloat16)
    din("sgnsinT", (HD, S), dt.bfloat16)
    din("ones_col", (128, 1))
    for l in range(depth):
        din(f"qkvwT_{l}", (H, 3 * DR), dt.bfloat16)
        din(f"qkvb_{l}", (HD, 6))
        din(f"projwT_{l}", (H, H), dt.bfloat16)
        din(f"projb_{l}", (128, HT))
        din(f"ln1w_{l}", (128, HT))
        din(f"ln1b_{l}", (128, HT))
        din(f"ln2w_{l}", (128, HT))
        din(f"ln2b_{l}", (128, HT))
        din(f"fc1wT_{l}", (H, IR), dt.bfloat16)
        din(f"fc1b_{l}", (128, IT))
        din(f"fc2wT_{l}", (IR, H), dt.bfloat16)
        din(f"fc2b_{l}", (128, HT))
    din("mnw", (128, HT))
    din("mnb", (128, HT))
    din("mfc1wT", (MH, IR))
    din("mfc1b", (128, IT))
    din("mfc2wT", (IR, OUT))
    din("mfc2b", (128, OT))

    out_r = nc.dram_tensor("out_r", [OUT, GR], dt.float32,
                           kind="ExternalOutput").ap()

    with tile.TileContext(nc) as tc:
        with tc.tile_pool(name="pers", bufs=1) as pers, \
             tc.tile_pool(name="sbuf", bufs=2) as sb, \
             tc.tile_pool(name="wpool", bufs=2) as wp, \
             tc.tile_pool(name="tiny", bufs=2) as tiny, \
             tc.tile_pool(name="bcast", bufs=2) as bc, \
             tc.tile_pool(name="psum", bufs=6, space="PSUM") as ps, \
             tc.tile_pool(name="dram", bufs=2, space="DRAM") as dram:
            pools = {"sbuf": sb, "psum": ps, "tiny": tiny, "bcast": bc}

            xT = pers.tile([128, HT * SR], dt.float32)
            cosT = pers.tile([HD, S], dt.bfloat16)
            sgnsinT = pers.tile([HD, S], dt.bfloat16)
            ident = pers.tile([128, 128], dt.bfloat16)
            onesr = pers.tile([128, 1], dt.float32r)
            onesb = pers.tile([128, 1], dt.bfloat16)
            nc.vector.memset(onesb[:], 1.0)
            nc.sync.dma_start(
                xT[:].rearrange("p (t s) -> p t s", t=HT),
                inp["x0"].rearrange("(t p) s -> p t s", t=HT))
            nc.sync.dma_start(cosT[:], inp["cosT"])
            nc.sync.dma_start(sgnsinT[:], inp["sgnsinT"])
            nc.sync.dma_start(onesr[:], inp["ones_col"].bitcast(dt.float32r))
            make_identity(nc, ident[:])

            for l in range(depth):
                # ---- per-layer small params ----
                lw = {}
                for nm in (f"ln1w_{l}", f"ln1b_{l}", f"ln2w_{l}", f"ln2b_{l}",
                           f"projb_{l}", f"fc1b_{l}", f"fc2b_{l}", f"qkvb_{l}"):
                    shp = list(inp[nm].shape)
                    t = tiny.tile(shp, dt.float32, tag="lw_" + nm.split("_")[0],
                                  bufs=2, name=f"t_{nm}")
                    nc.sync.dma_start(t[:], inp[nm])
                    lw[nm] = t

                # ---- LN1 -> bf16 -> AllGather ----
                lno = sb.tile([128, HT * SR], dt.bfloat16, tag="lnout", bufs=1,
                              name=f"ln1o_{l}")
                _layernorm(nc, pools, xT[:], lw[f"ln1w_{l}"][:],
                           lw[f"ln1b_{l}"][:], lno[:], onesr, f"l1_{l}")
                agi = dram.tile([H, SR], dt.bfloat16, tag="ag_in",
                                name=f"agi1_{l}")
                nc.sync.dma_start(
                    agi[:].rearrange("(t p) s -> p t s", t=HT),
                    lno[:].rearrange("p (t s) -> p t s", t=HT))
                ago = dram.tile([NCORE * H, SR], dt.bfloat16,
                                addr_space="Shared", tag="ag_out",
                                name=f"ago1_{l}")
                nc.gpsimd.collective_compute(
                    "AllGather", OP.bypass, replica_groups=RG,
                    ins=[agi[:].opt()], outs=[ago[:].opt()])

                # ---- qkv matmul (dim-major out, 6 m-slices of 80) ----
                wq = []
                for k in range(HT):
                    w = wp.tile([128, 3 * DR], dt.bfloat16, tag="wqkv",
                                bufs=HT, name=f"wq_{l}_{k}")
                    nc.sync.dma_start(
                        w[:], inp[f"qkvwT_{l}"][k * 128:(k + 1) * 128, :])
                    wq.append(w)
                qkvt = [sb.tile([HD, S], dt.bfloat16, tag=f"qkvt{i}", bufs=1,
                                name=f"qkvt{i}_{l}") for i in range(6)]
                agov = ago[:].rearrange("(r k p) s -> k p r s", r=NCORE, k=HT)
                for n in range(4):
                    rhs = []
                    for k in range(HT):
                        r = sb.tile([128, 512], dt.bfloat16, tag="rhs_h1",
                                    bufs=4, name=f"rhs1_{l}_{n}_{k}")
                        nc.sync.dma_start(
                            r[:].rearrange("p (r s) -> p r s", r=2),
                            agov[k][:, 2 * n:2 * n + 2, :])
                        rhs.append(r)
                    for m in range(6):
                        acc = ps.tile([HD, 512], dt.float32, tag="mm512",
                                      bufs=6, name=f"qkvps_{l}_{n}_{m}")
                        for k in range(HT):
                            nc.tensor.matmul(
                                acc[:], wq[k][:, m * HD:(m + 1) * HD], rhs[k][:],
                                start=(k == 0), stop=(k == HT - 1))
                        nc.scalar.activation(
                            qkvt[m][:, n * 512:(n + 1) * 512], acc[:],
                            AF.Identity, bias=lw[f"qkvb_{l}"][:, m:m + 1])

                # ---- rope on q0,q1,k0,k1 (in-place, bf16) ----
                for i in range(4):
                    rot = sb.tile([HD, S], dt.bfloat16, tag="rot", bufs=1,
                                  name=f"rot{i}_{l}")
                    nc.sync.dma_start(rot[0:HD // 2, :], qkvt[i][HD // 2:HD, :])
                    nc.sync.dma_start(rot[HD // 2:HD, :], qkvt[i][0:HD // 2, :])
                    nc.vector.tensor_tensor(qkvt[i][:], qkvt[i][:], cosT[:],
                                            OP.mult)
                    nc.vector.tensor_tensor(rot[:], rot[:], sgnsinT[:], OP.mult)
                    nc.vector.tensor_tensor(qkvt[i][:], qkvt[i][:], rot[:],
                                            OP.add)

                # ---- attention per head ----
                aoT = [sb.tile([HD, S], dt.bfloat16, tag=f"aoT{h}", bufs=1,
                               name=f"aoT{h}_{l}") for h in range(HPC)]
                for h in range(HPC):
                    qb, kb, vr = qkvt[h], qkvt[2 + h], qkvt[4 + h]
                    v1 = []
                    for t in range(ST):
                        v1t = sb.tile([128, HD], dt.bfloat16, tag="v1",
                                      bufs=ST + 1, name=f"v1_{l}_{h}_{t}")
                        tp = ps.tile([128, 512], dt.bfloat16, tag="mm512",
                                     bufs=6, name=f"vtp_{l}_{h}_{t}")
                        nc.tensor.transpose(
                            tp[0:128, 0:HD], vr[:, t * 128:(t + 1) * 128],
                            ident[0:HD, 0:HD])
                        nc.scalar.activation(v1t[:], tp[0:128, 0:HD], AF.Copy)
                        v1.append(v1t)
                    sums = tiny.tile([1, S], dt.float32, tag="sums", bufs=1,
                                     name=f"sums_{l}_{h}")
                    for sblk in range(4):
                        av = ps.tile([128, 512], dt.float32, tag="mm512",
                                     bufs=6, name=f"avps_{l}_{h}_{sblk}")
                        smp = ps.tile([1, 512], dt.float32, tag="stat",
                                      bufs=2, name=f"smp_{l}_{h}_{sblk}")
                        for t in range(ST):
                            sc = ps.tile([128, 512], dt.float32, tag="mm512",
                                         bufs=6, name=f"scps_{l}_{h}_{sblk}_{t}")
                            nc.tensor.matmul(
                                sc[:], kb[:, t * 128:(t + 1) * 128],
                                qb[:, sblk * 512:(sblk + 1) * 512],
                                start=True, stop=True)
                            ex = sb.tile([128, 512], dt.bfloat16, tag="exp",
                                         bufs=4, name=f"ex_{l}_{h}_{sblk}_{t}")
                            nc.scalar.activation(ex[:], sc[:], AF.Exp,
                                                 scale=SCALE)
                            nc.tensor.matmul(
                                av[0:HD, :], v1[t][:], ex[:],
                                start=(t == 0), stop=(t == ST - 1))
                            nc.tensor.matmul(
                                smp[:], onesb[:], ex[:],
                                start=(t == 0), stop=(t == ST - 1))
                        nc.scalar.activation(
                            aoT[h][:, sblk * 512:(sblk + 1) * 512],
                            av[0:HD, :], AF.Copy)
                        nc.vector.tensor_copy(
                            sums[:, sblk * 512:(sblk + 1) * 512], smp[:])
                    nc.vector.reciprocal(sums[:], sums[:])
                    rsumb = tiny.tile([1, S], dt.bfloat16, tag="rsumb", bufs=1,
                                      name=f"rsumb_{l}_{h}")
                    nc.scalar.activation(rsumb[:], sums[:], AF.Copy)
                    rsb = bc.tile([HD, S], dt.bfloat16, tag="rsb", bufs=1,
                                  name=f"rsb_{l}_{h}")
                    nc.gpsimd.partition_broadcast(rsb[:], rsumb[:])
                    nc.vector.tensor_tensor(aoT[h][:], aoT[h][:], rsb[:],
                                            OP.mult)

                # ---- AllToAll attn out (bf16) ----
                ai = dram.tile([H, SR], dt.bfloat16, tag="a2a_in",
                               name=f"a2ai_{l}")
                for j in range(NCORE):
                    for h in range(HPC):
                        nc.sync.dma_start(
                            ai[j * DR + h * HD: j * DR + (h + 1) * HD, :],
                            aoT[h][:, j * SR:(j + 1) * SR])
                ao = dram.tile([H, SR], dt.bfloat16, tag="a2a_out",
                               name=f"a2ao_{l}")
                nc.gpsimd.collective_compute(
                    "AllToAll", OP.bypass, replica_groups=RG,
                    ins=[ai[:].opt()], outs=[ao[:].opt()])

                # ---- proj + residual (bf16 matmul, fp32 accumulate) ----
                prj = []
                for k in range(HT):
                    r = sb.tile([128, SR], dt.bfloat16, tag="prj", bufs=HT,
                                name=f"prj_{l}_{k}")
                    nc.sync.dma_start(r[:], ao[k * 128:(k + 1) * 128, :])
                    prj.append(r)
                for m in range(HT):
                    acc = ps.tile([128, 512], dt.float32, tag="mm512", bufs=6,
                                  name=f"pps_{l}_{m}")
                    for k in range(HT):
                        w = wp.tile([128, 128], dt.bfloat16, tag="wproj",
                                    bufs=4, name=f"wp_{l}_{m}_{k}")
                        nc.sync.dma_start(
                            w[:], inp[f"projwT_{l}"]
                            [k * 128:(k + 1) * 128, m * 128:(m + 1) * 128])
                        nc.tensor.matmul(acc[0:128, 0:SR], w[:], prj[k][:],
                                         start=(k == 0), stop=(k == HT - 1))
                    xs = xT[:, m * SR:(m + 1) * SR]
                    nc.vector.tensor_tensor(xs, xs, acc[0:128, 0:SR], OP.add)
                    nc.vector.tensor_scalar_add(xs, xs,
                                                lw[f"projb_{l}"][:, m:m + 1])

                # ---- LN2 -> bf16 -> AllGather ----
                ln2 = sb.tile([128, HT * SR], dt.bfloat16, tag="lnout", bufs=1,
                              name=f"ln2o_{l}")
                _layernorm(nc, pools, xT[:], lw[f"ln2w_{l}"][:],
                           lw[f"ln2b_{l}"][:], ln2[:], onesr, f"l2_{l}")
                agi2 = dram.tile([H, SR], dt.bfloat16, tag="ag_in",
                                 name=f"agi2_{l}")
                nc.sync.dma_start(
                    agi2[:].rearrange("(t p) s -> p t s", t=HT),
                    ln2[:].rearrange("p (t s) -> p t s", t=HT))
                ago2 = dram.tile([NCORE * H, SR], dt.bfloat16,
                                 addr_space="Shared", tag="ag_out",
                                 name=f"ago2_{l}")
                nc.gpsimd.collective_compute(
                    "AllGather", OP.bypass, replica_groups=RG,
                    ins=[agi2[:].opt()], outs=[ago2[:].opt()])

                # ---- fc1 + gelu (bf16) ----
                wf1 = []
                for k in range(HT):
                    w = wp.tile([128, IR], dt.bfloat16, tag="wfc1",
                                bufs=HT, name=f"wf1_{l}_{k}")
                    nc.sync.dma_start(
                        w[:], inp[f"fc1wT_{l}"][k * 128:(k + 1) * 128, :])
                    wf1.append(w)
                gT = [sb.tile([128, S], dt.bfloat16, tag=f"gT{i}", bufs=1,
                              name=f"gT{i}_{l}") for i in range(IT)]
                ago2v = ago2[:].rearrange("(r k p) s -> k p r s", r=NCORE, k=HT)
                for n in range(4):
                    rhs = []
                    for k in range(HT):
                        r = sb.tile([128, 512], dt.bfloat16, tag="rhs_h1",
                                    bufs=4, name=f"rhs2_{l}_{n}_{k}")
                        nc.sync.dma_start(
                            r[:].rearrange("p (r s) -> p r s", r=2),
                            ago2v[k][:, 2 * n:2 * n + 2, :])
                        rhs.append(r)
                    for m in range(IT):
                        acc = ps.tile([128, 512], dt.float32, tag="mm512",
                                      bufs=6, name=f"f1ps_{l}_{n}_{m}")
                        for k in range(HT):
                            nc.tensor.matmul(
                                acc[:], wf1[k][:, m * 128:(m + 1) * 128],
                                rhs[k][:], start=(k == 0), stop=(k == HT - 1))
                        nc.scalar.activation(
                            gT[m][:, n * 512:(n + 1) * 512], acc[:], AF.Gelu,
                            bias=lw[f"fc1b_{l}"][:, m:m + 1])

                # ---- fc2 partial -> ReduceScatter (fp32) ----
                rsi = dram.tile([NCORE * H, SR], dt.bfloat16, tag="rs_in",
                                name=f"rsi_{l}")
                rsiv = rsi[:].rearrange("(j t p) s -> j t p s", j=NCORE, t=HT)
                for m in range(HT):
                    accs = [ps.tile([128, 512], dt.float32, tag="mm512",
                                    bufs=6, name=f"f2ps_{l}_{m}_{n}")
                            for n in range(4)]
                    for k in range(IT):
                        w = wp.tile([128, 128], dt.bfloat16, tag="wfc2",
                                    bufs=4, name=f"wf2_{l}_{m}_{k}")
                        nc.sync.dma_start(
                            w[:], inp[f"fc2wT_{l}"]
                            [k * 128:(k + 1) * 128, m * 128:(m + 1) * 128])
                        for n in range(4):
                            nc.tensor.matmul(
                                accs[n][:], w[:],
                                gT[k][:, n * 512:(n + 1) * 512],
                                start=(k == 0), stop=(k == IT - 1))
                    for n in range(4):
                        ev = sb.tile([128, 512], dt.bfloat16, tag="f2ev",
                                     bufs=2, name=f"f2ev_{l}_{m}_{n}")
                        nc.scalar.activation(ev[:], accs[n][:], AF.Copy)
                        nc.sync.dma_start(
                            rsiv[2 * n:2 * n + 2, m].transpose([1, 0, 2]),
                            ev[:].rearrange("p (j s) -> p j s", j=2))
                rso = dram.tile([H, SR], dt.bfloat16, tag="rs_out",
                                name=f"rso_{l}")
                nc.gpsimd.collective_compute(
                    "ReduceScatter", OP.add, replica_groups=RG,
                    ins=[rsi[:].opt()], outs=[rso[:].opt()])
                for m in range(HT):
                    r = sb.tile([128, SR], dt.bfloat16, tag="rsout", bufs=3,
                                name=f"rsov_{l}_{m}")
                    nc.sync.dma_start(r[:], rso[m * 128:(m + 1) * 128, :])
                    xs = xT[:, m * SR:(m + 1) * SR]
                    nc.vector.tensor_tensor(xs, xs, r[:], OP.add)
                    nc.vector.tensor_scalar_add(xs, xs,
                                                lw[f"fc2b_{l}"][:, m:m + 1])

            # ================= merger =================
            MPART = int(os.environ.get("KERNEL_MERGER_PART", "9"))
            if MPART == 0:
                zt = sb.tile([128, GR], dt.float32, tag="mout", bufs=3)
                nc.vector.memset(zt[:], 0.0)
                for m in range(OT):
                    nc.sync.dma_start(out_r[m * 128:(m + 1) * 128, :], zt[:])
            else:
                mnw = tiny.tile([128, HT], dt.float32, tag="lw_ln1w", bufs=2)
                mnb = tiny.tile([128, HT], dt.float32, tag="lw_ln1b", bufs=2)
                nc.sync.dma_start(mnw[:], inp["mnw"])
                nc.sync.dma_start(mnb[:], inp["mnb"])
                mln = sb.tile([128, HT * SR], dt.float32, tag="mln", bufs=1)
                _layernorm(nc, pools, xT[:], mnw[:], mnb[:], mln[:], onesr, "mn")
                ysb = sb.tile([128, HT * SR], dt.float32, tag="ln_xsq", bufs=2)
                for j in range(4):
                    nc.vector.tensor_copy(
                        ysb[:].rearrange("p (j t g) -> j p t g", j=4, t=HT)[j],
                        mln[:].rearrange("p (t g j) -> j p t g", t=HT, j=4)[j])
                yagi = dram.tile([MH, GR], dt.float32, tag="y_agi")
                nc.sync.dma_start(
                    yagi[:].rearrange("(j t p) g -> p (j t) g", j=4, t=HT),
                    ysb[:].rearrange("p (j t g) -> p (j t) g", j=4, t=HT))
                yago = dram.tile([NCORE * MH, GR], dt.float32, addr_space="Shared",
                                 tag="y_ago")
                nc.gpsimd.collective_compute(
                    "AllGather", OP.bypass, replica_groups=RG,
                    ins=[yagi[:].opt()], outs=[yago[:].opt()])

                mfc1b = tiny.tile([128, IT], dt.float32, tag="lw_fc1b", bufs=2)
                nc.sync.dma_start(mfc1b[:], inp["mfc1b"])
                mg = [sb.tile([128, NCORE * GR], dt.float32r, tag=f"gT{i}", bufs=1,
                              name=f"mg{i}") for i in range(IT)]
                yagov = yago[:].rearrange("(r k p) g -> k p r g", r=NCORE, k=MT)
                maccs = [ps.tile([128, 512], dt.float32, tag="mm512", bufs=6,
                                 name=f"m1ps_{m}") for m in range(IT)]
                for k in range(MT):
                    w = wp.tile([128, IR], dt.float32r, tag="mw1", bufs=2,
                                name=f"mw1_{k}")
                    nc.sync.dma_start(
                        w[:], inp["mfc1wT"][k * 128:(k + 1) * 128, :]
                        .bitcast(dt.float32r))
                    r = sb.tile([128, 512], dt.float32r, tag="yrhs", bufs=2,
                                name=f"mrhs_{k}")
                    nc.sync.dma_start(r[:].rearrange("p (r g) -> p r g", r=NCORE),
                                      yagov[k].bitcast(dt.float32r))
                    for m in range(IT):
                        nc.tensor.matmul(maccs[m][:],
                                         w[:, m * 128:(m + 1) * 128], r[:],
                                         start=(k == 0), stop=(k == MT - 1))
                for m in range(IT):
                    nc.scalar.activation(mg[m][:], maccs[m][:], AF.Gelu,
                                         bias=mfc1b[:, m:m + 1])

                mrsi = dram.tile([NCORE * OUT, GR], dt.bfloat16, tag="mrs_in")
                mrsiv = mrsi[:].rearrange("(j t p) g -> j t p g", j=NCORE, t=OT)
                for m in range(OT):
                    acc = ps.tile([128, 512], dt.float32, tag="mm512", bufs=6,
                                  name=f"m2ps_{m}")
                    for k in range(IT):
                        w = wp.tile([128, 128], dt.float32r, tag="mw2", bufs=2,
                                    name=f"mw2_{m}_{k}")
                        nc.sync.dma_start(
                            w[:], inp["mfc2wT"]
                            [k * 128:(k + 1) * 128, m * 128:(m + 1) * 128]
                            .bitcast(dt.float32r))
                        nc.tensor.matmul(acc[:], w[:], mg[k][:],
                                         start=(k == 0), stop=(k == IT - 1))
                    ev = sb.tile([128, 512], dt.bfloat16, tag="f2ev", bufs=2,
                                 name=f"m2ev_{m}")
                    nc.scalar.activation(ev[:], acc[:], AF.Copy)
                    nc.sync.dma_start(
                        mrsiv[:, m].transpose([1, 0, 2]),
                        ev[:].rearrange("p (j g) -> p j g", j=NCORE))
                mrso = dram.tile([OUT, GR], dt.bfloat16, tag="mrs_out")
                nc.gpsimd.collective_compute(
                    "ReduceScatter", OP.add, replica_groups=RG,
                    ins=[mrsi[:].opt()], outs=[mrso[:].opt()])
                mfc2b = tiny.tile([128, OT], dt.float32, tag="lw_m2b", bufs=1)
                nc.sync.dma_start(mfc2b[:], inp["mfc2b"])
                for m in range(OT):
                    r = sb.tile([128, GR], dt.bfloat16, tag="rsout", bufs=3,
                                name=f"mro_{m}")
                    nc.sync.dma_start(r[:], mrso[m * 128:(m + 1) * 128, :])
                    o = sb.tile([128, GR], dt.float32, tag="mout", bufs=3,
                                name=f"mo_{m}")
                    nc.scalar.activation(o[:], r[:], AF.Identity,
                                         bias=mfc2b[:, m:m + 1])
                    nc.sync.dma_start(out_r[m * 128:(m + 1) * 128, :], o[:])

    nc.compile()
    return nc


def _prep_inputs(inputs, depth=DEPTH):
    f32 = lambda a: np.ascontiguousarray(np.asarray(a), dtype=np.float32)
    bf = lambda a: np.ascontiguousarray(np.asarray(a, dtype=np.float32)
                                        .astype(BF))
    hs = f32(inputs["hidden_states"])
    cos, sin = f32(inputs["cos"]), f32(inputs["sin"])
    sgnsin = np.concatenate([-sin[:, :HD // 2], sin[:, HD // 2:]], axis=1)
    qkv_w, qkv_b = f32(inputs["qkv_w"]), f32(inputs["qkv_b"])
    proj_w = f32(inputs["proj_w"])
    fc1_w, fc2_w = f32(inputs["fc1_w"]), f32(inputs["fc2_w"])

    def fold(vec, cols):
        return np.ascontiguousarray(f32(vec).reshape(cols, -1).T)

    in_maps = []
    for c in range(NCORE):
        m = {
            "x0": np.ascontiguousarray(hs[c * SR:(c + 1) * SR].T),
            "cosT": bf(cos.T),
            "sgnsinT": bf(sgnsin.T),
            "ones_col": np.ones((128, 1), np.float32),
        }
        h0, h1 = HPC * c, HPC * c + 1
        for l in range(depth):
            rows, bias = [], []
            for part in range(3):  # q, k, v
                for h in (h0, h1):
                    rows.append(qkv_w[l][part * H + h * HD:
                                         part * H + (h + 1) * HD, :])
                    bias.append(qkv_b[l][part * H + h * HD:
                                         part * H + (h + 1) * HD])
            m[f"qkvwT_{l}"] = bf(np.concatenate(rows, axis=0).T)
            m[f"qkvb_{l}"] = np.ascontiguousarray(np.stack(bias, axis=1))
            m[f"projwT_{l}"] = bf(proj_w[l].T)
            m[f"projb_{l}"] = fold(inputs["proj_b"][l], HT)
            m[f"ln1w_{l}"] = fold(inputs["ln1_w"][l], HT)
            m[f"ln1b_{l}"] = fold(inputs["ln1_b"][l], HT)
            m[f"ln2w_{l}"] = fold(inputs["ln2_w"][l], HT)
            m[f"ln2b_{l}"] = fold(inputs["ln2_b"][l], HT)
            m[f"fc1wT_{l}"] = bf(fc1_w[l][c * IR:(c + 1) * IR, :].T)
            m[f"fc1b_{l}"] = fold(np.asarray(inputs["fc1_b"])[l][c * IR:(c + 1) * IR], IT)
            m[f"fc2wT_{l}"] = bf(fc2_w[l][:, c * IR:(c + 1) * IR].T)
            m[f"fc2b_{l}"] = fold(inputs["fc2_b"][l], HT)
        m["mnw"] = fold(inputs["mnorm_w"], HT)
        m["mnb"] = fold(inputs["mnorm_b"], HT)
        m["mfc1wT"] = np.ascontiguousarray(
            f32(inputs["mfc1_w"])[c * IR:(c + 1) * IR, :].T)
        m["mfc1b"] = fold(f32(inputs["mfc1_b"])[c * IR:(c + 1) * IR], IT)
        m["mfc2wT"] = np.ascontiguousarray(
            f32(inputs["mfc2_w"])[:, c * IR:(c + 1) * IR].T)
        m["mfc2b"] = fold(inputs["mfc2_b"], OT)
        in_maps.append(m)
    return in_maps


def kernel(**inputs) -> np.ndarray:
    if "nc" not in _cached:
        _cached["nc"] = build_nc()
    nc = _cached["nc"]
    in_maps = _prep_inputs(inputs)
    res = run_bass_kernel_spmd(
        nc, in_maps, core_ids=list(range(NCORE)),
        trace=bool(int(os.environ.get("KERNEL_TRACE", "0"))))
    _cached["last_result"] = res
    out = np.empty((S // 4, OUT), np.float32)
    for c in range(NCORE):
        out[c * GR:(c + 1) * GR, :] = res.results[c]["out_r"].T
    return out



# revision 28
# speedup vs baseline: 1.5565x; 1.5565x over previous
"""Trainium2 Bass kernel for nn_CPUVisionModel (8-block ViT + merger MLP).

Sharding: sequence-parallel residual stream (256 tokens/core, dim-major
(hidden, token) layout) + head-sharded attention (2 heads/core) +
sequence-local MLP (full fc1/fc2 weights streamed from HBM, no
collective).  Collectives per layer: AllGather(LN1 out, bf16) +
AllToAll(attn out, bf16) only.  Merger: AllGather(y, bf16) +
ReduceScatter(out, bf16).

All matmuls run in bf16 (fp32 PSUM accumulation); residual stream,
layernorms and reductions in fp32.  The softmax denominator rides in
the AV matmul via a ones column at packed row 96 (PSUM partition bases
must be 0/32/64/96).
"""
import os
import sys

sys.path.insert(0, "/opt/trn_rl_repo")

import ml_dtypes
import numpy as np

import concourse.bass as bass
import concourse.mybir as mybir
import concourse.tile as tile
from concourse import bacc
from concourse.bass_utils import run_bass_kernel_spmd
from concourse.masks import make_identity

dt = mybir.dt
AF = mybir.ActivationFunctionType
OP = mybir.AluOpType
BF = ml_dtypes.bfloat16

# model dims
S, H, NH, HD, I = 2048, 1280, 16, 80, 5120
MH, OUT = 5120, 3584
DEPTH = int(os.environ.get("KERNEL_DEPTH", "8"))
SCALE = HD ** -0.5
EPS = 1e-6

NCORE = 8
SR = S // NCORE          # 256 tokens per core
HPC = NH // NCORE        # 2 heads per core
DR = HPC * HD            # 160 attn dims per core
IR = I // NCORE          # 640 intermediate per core
GR = SR // 4             # 64 merger rows per core
HT = H // 128            # 10 hidden k-tiles
IT = IR // 128           # 5 intermediate tiles per core
ST = S // 128            # 16 token tiles
OT = OUT // 128          # 28 output tiles
MT = MH // 128           # 40 merger-hidden tiles
RG = [list(range(NCORE))]

_cached = {}


def _layernorm(nc, pools, src, wln, bln, dst, onesr, name):
    """LayerNorm over hidden axis of dim-major src (128, HT*SR) -> dst."""
    sb, ps, tiny, bc = pools["sbuf"], pools["psum"], pools["tiny"], pools["bcast"]
    xr = sb.tile([128, HT * SR], dt.float32r, tag="ln_xr", bufs=1,
                 name=f"xr_{name}")
    xsq = sb.tile([128, HT * SR], dt.float32r, tag="ln_xsq", bufs=1,
                  name=f"xsq_{name}")
    nc.scalar.activation(xr[:], src, AF.Copy)
    nc.scalar.activation(xsq[:], src, AF.Square)
    ssum = ps.tile([1, SR], dt.float32, tag="stat", bufs=2, name=f"ss_{name}")
    qsum = ps.tile([1, SR], dt.float32, tag="stat", bufs=2, name=f"qs_{name}")
    for t in range(HT):
        nc.tensor.matmul(ssum[:], onesr[:], xr[:][:, t * SR:(t + 1) * SR],
                         start=(t == 0), stop=(t == HT - 1))
    for t in range(HT):
        nc.tensor.matmul(qsum[:], onesr[:], xsq[:, t * SR:(t + 1) * SR],
                         start=(t == 0), stop=(t == HT - 1))
    mean = tiny.tile([1, SR], dt.float32, tag="t_mean", bufs=2, name=f"mn_{name}")
    var = tiny.tile([1, SR], dt.float32, tag="t_var", bufs=2, name=f"vr_{name}")
    std = tiny.tile([1, SR], dt.float32, tag="t_std", bufs=2, name=f"sd_{name}")
    rstd = tiny.tile([1, SR], dt.float32, tag="t_rstd", bufs=2, name=f"rs_{name}")
    msq = tiny.tile([1, SR], dt.float32, tag="t_msq", bufs=2, name=f"mq_{name}")
    nc.vector.tensor_scalar_mul(mean[:], ssum[:], 1.0 / H)
    nc.vector.tensor_scalar_mul(var[:], qsum[:], 1.0 / H)
    nc.vector.tensor_tensor(msq[:], mean[:], mean[:], OP.mult)
    nc.vector.tensor_tensor(var[:], var[:], msq[:], OP.subtract)
    nc.vector.tensor_scalar_add(var[:], var[:], EPS)
    nc.scalar.activation(std[:], var[:], AF.Sqrt)
    nc.vector.reciprocal(rstd[:], std[:])
    meanb = bc.tile([128, SR], dt.float32, tag="b_mean", bufs=2, name=f"mb_{name}")
    rstdb = bc.tile([128, SR], dt.float32, tag="b_rstd", bufs=2, name=f"rb_{name}")
    nc.gpsimd.partition_broadcast(meanb[:], mean[:])
    nc.gpsimd.partition_broadcast(rstdb[:], rstd[:])
    tmp = sb.tile([128, HT * SR], dt.float32, tag="ln_xsq", bufs=1,
                  name=f"lt_{name}")
    srcv = src.rearrange("p (t s) -> p t s", t=HT)
    tmpv = tmp[:].rearrange("p (t s) -> p t s", t=HT)
    dstv = dst.rearrange("p (t s) -> p t s", t=HT)
    mb = meanb[:].unsqueeze(1).broadcast_to((128, HT, SR))
    rb = rstdb[:].unsqueeze(1).broadcast_to((128, HT, SR))
    wv = wln.unsqueeze(2).broadcast_to((128, HT, SR))
    bv = bln.unsqueeze(2).broadcast_to((128, HT, SR))
    nc.vector.tensor_tensor(tmpv, srcv, mb, OP.subtract)
    nc.vector.tensor_tensor(tmpv, tmpv, rb, OP.mult)
    nc.vector.tensor_tensor(tmpv, tmpv, wv, OP.mult)
    nc.vector.tensor_tensor(dstv, tmpv, bv, OP.add)


def build_nc(depth=DEPTH, num_cores=NCORE):
    nc = bacc.Bacc("TRN2", target_bir_lowering=False, debug=False,
                   enable_asserts=True, num_devices=num_cores)

    inp = {}

    def din(name, shape, dd=dt.float32):
        inp[name] = nc.dram_tensor(name, list(shape), dd,
                                   kind="ExternalInput").ap()
        return inp[name]

    din("x0", (H, SR))
    din("cosT", (HD, S), dt.bfloat16)
    din("sgnsinT", (HD, S), dt.bfloat16)
    din("ones_col", (128, 1))
    for l in range(depth):
        din(f"qkvwT_{l}", (H, 3 * DR), dt.bfloat16)
        din(f"qkvb_{l}", (HD, 6))
        din(f"projwT_{l}", (H, H), dt.bfloat16)
        din(f"projb_{l}", (128, HT))
        din(f"ln1w_{l}", (128, HT))
        din(f"ln1b_{l}", (128, HT))
        din(f"ln2w_{l}", (128, HT))
        din(f"ln2b_{l}", (128, HT))
        din(f"fc1wT_{l}", (H, I), dt.bfloat16)
        din(f"fc1b_{l}", (128, I // 128))
        din(f"fc2wT_{l}", (I, H), dt.bfloat16)
        din(f"fc2b_{l}", (128, HT))
    din("mnw", (128, HT))
    din("mnb", (128, HT))
    din("mfc1wT", (MH, IR), dt.bfloat16)
    din("mfc1b", (128, IT))
    din("mfc2wT", (IR, OUT), dt.bfloat16)
    din("mfc2b", (128, OT))

    out_r = nc.dram_tensor("out_r", [OUT, GR], dt.float32,
                           kind="ExternalOutput").ap()

    with tile.TileContext(nc) as tc:
        with tc.tile_pool(name="pers", bufs=1) as pers, \
             tc.tile_pool(name="sbuf", bufs=2) as sb, \
             tc.tile_pool(name="wpool", bufs=2) as wp, \
             tc.tile_pool(name="tiny", bufs=2) as tiny, \
             tc.tile_pool(name="bcast", bufs=2) as bc, \
             tc.tile_pool(name="psum", bufs=6, space="PSUM") as ps, \
             tc.tile_pool(name="dram", bufs=2, space="DRAM") as dram:
            pools = {"sbuf": sb, "psum": ps, "tiny": tiny, "bcast": bc}

            xT = pers.tile([128, HT * SR], dt.float32)
            cosT = pers.tile([HD, S], dt.bfloat16)
            sgnsinT = pers.tile([HD, S], dt.bfloat16)
            ident = pers.tile([128, 128], dt.bfloat16)
            onesr = pers.tile([128, 1], dt.float32r)
            onesb = pers.tile([128, 1], dt.bfloat16)
            nc.vector.memset(onesb[:], 1.0)
            nc.sync.dma_start(
                xT[:].rearrange("p (t s) -> p t s", t=HT),
                inp["x0"].rearrange("(t p) s -> p t s", t=HT))
            nc.sync.dma_start(cosT[:], inp["cosT"])
            nc.sync.dma_start(sgnsinT[:], inp["sgnsinT"])
            nc.sync.dma_start(onesr[:], inp["ones_col"].bitcast(dt.float32r))
            make_identity(nc, ident[:])

            for l in range(depth):
                # ---- per-layer small params ----
                lw = {}
                for nm in (f"ln1w_{l}", f"ln1b_{l}", f"ln2w_{l}", f"ln2b_{l}",
                           f"projb_{l}", f"fc1b_{l}", f"fc2b_{l}", f"qkvb_{l}"):
                    shp = list(inp[nm].shape)
                    t = tiny.tile(shp, dt.float32, tag="lw_" + nm.split("_")[0],
                                  bufs=2, name=f"t_{nm}")
                    nc.sync.dma_start(t[:], inp[nm])
                    lw[nm] = t

                # ---- LN1 -> fp8 -> AllGather ----
                lno = sb.tile([128, HT * SR], dt.bfloat16, tag="lnout", bufs=1,
                              name=f"ln1o_{l}")
                _layernorm(nc, pools, xT[:], lw[f"ln1w_{l}"][:],
                           lw[f"ln1b_{l}"][:], lno[:], onesr, f"l1_{l}")
                agi = dram.tile([H, SR], dt.bfloat16, tag="ag_in",
                                name=f"agi1_{l}")
                nc.sync.dma_start(
                    agi[:].rearrange("(t p) s -> p t s", t=HT),
                    lno[:].rearrange("p (t s) -> p t s", t=HT))
                ago = dram.tile([NCORE * H, SR], dt.bfloat16,
                                addr_space="Shared", tag="ag_out",
                                name=f"ago1_{l}")
                nc.gpsimd.collective_compute(
                    "AllGather", OP.bypass, replica_groups=RG,
                    ins=[agi[:].opt()], outs=[ago[:].opt()])

                # ---- qkv matmul (dim-major out, 6 m-slices of 80) ----
                wq = []
                for k in range(HT):
                    w = wp.tile([128, 3 * DR], dt.bfloat16, tag="wqkv",
                                bufs=HT, name=f"wq_{l}_{k}")
                    nc.sync.dma_start(
                        w[:], inp[f"qkvwT_{l}"][k * 128:(k + 1) * 128, :])
                    wq.append(w)
                qkvt = [sb.tile([HD, S], dt.bfloat16, tag=f"qkvt{i}", bufs=1,
                                name=f"qkvt{i}_{l}") for i in range(6)]
                agov = ago[:].rearrange("(r k p) s -> k p r s", r=NCORE, k=HT)
                for n in range(4):
                    rhs = []
                    for k in range(HT):
                        r = sb.tile([128, 512], dt.bfloat16, tag="rhs_h1",
                                    bufs=4, name=f"rhs1_{l}_{n}_{k}")
                        nc.sync.dma_start(
                            r[:].rearrange("p (r s) -> p r s", r=2),
                            agov[k][:, 2 * n:2 * n + 2, :])
                        rhs.append(r)
                    for m in range(6):
                        acc = ps.tile([HD, 512], dt.float32, tag="mm512",
                                      bufs=6, name=f"qkvps_{l}_{n}_{m}")
                        for k in range(HT):
                            nc.tensor.matmul(
                                acc[:], wq[k][:, m * HD:(m + 1) * HD], rhs[k][:],
                                start=(k == 0), stop=(k == HT - 1))
                        nc.scalar.activation(
                            qkvt[m][:, n * 512:(n + 1) * 512], acc[:],
                            AF.Identity, bias=lw[f"qkvb_{l}"][:, m:m + 1])

                # ---- rope on q0,q1,k0,k1 (in-place, bf16) ----
                for i in range(4):
                    rot = sb.tile([HD, S], dt.bfloat16, tag="rot", bufs=1,
                                  name=f"rot{i}_{l}")
                    nc.sync.dma_start(rot[0:HD // 2, :], qkvt[i][HD // 2:HD, :])
                    nc.sync.dma_start(rot[HD // 2:HD, :], qkvt[i][0:HD // 2, :])
                    nc.vector.tensor_tensor(qkvt[i][:], qkvt[i][:], cosT[:],
                                            OP.mult)
                    nc.vector.tensor_tensor(rot[:], rot[:], sgnsinT[:], OP.mult)
                    nc.vector.tensor_tensor(qkvt[i][:], qkvt[i][:], rot[:],
                                            OP.add)

                # ---- attention per head ----
                aoT = [sb.tile([HD, S], dt.bfloat16, tag=f"aoT{h}", bufs=1,
                               name=f"aoT{h}_{l}") for h in range(HPC)]
                for h in range(HPC):
                    qb, kb, vr = qkvt[h], qkvt[2 + h], qkvt[4 + h]
                    v1 = []
                    for t in range(ST):
                        # [tokens, 97]: ones in columns 80:97 fold the softmax
                        # denominator into the AV matmul (psum partition base
                        # must be 0/32/64/96, so the sum row lives at 96)
                        v1t = sb.tile([128, 97], dt.bfloat16, tag="v1",
                                      bufs=ST + 1, name=f"v1_{l}_{h}_{t}")
                        tp = ps.tile([128, 512], dt.bfloat16, tag="mm512",
                                     bufs=6, name=f"vtp_{l}_{h}_{t}")
                        nc.tensor.transpose(
                            tp[0:128, 0:HD], vr[:, t * 128:(t + 1) * 128],
                            ident[0:HD, 0:HD])
                        nc.scalar.activation(v1t[:, 0:HD], tp[0:128, 0:HD],
                                             AF.Copy)
                        nc.vector.memset(v1t[:, HD:97], 1.0)
                        v1.append(v1t)
                    sums = tiny.tile([1, S], dt.float32, tag="sums", bufs=1,
                                     name=f"sums_{l}_{h}")
                    for sblk in range(4):
                        av = ps.tile([128, 512], dt.float32, tag="mm512",
                                     bufs=6, name=f"avps_{l}_{h}_{sblk}")
                        for t in range(ST):
                            sc = ps.tile([128, 512], dt.float32, tag="mm512",
                                         bufs=6, name=f"scps_{l}_{h}_{sblk}_{t}")
                            nc.tensor.matmul(
                                sc[:], kb[:, t * 128:(t + 1) * 128],
                                qb[:, sblk * 512:(sblk + 1) * 512],
                                start=True, stop=True)
                            ex = sb.tile([128, 512], dt.bfloat16, tag="exp",
                                         bufs=4, name=f"ex_{l}_{h}_{sblk}_{t}")
                            nc.scalar.activation(ex[:], sc[:], AF.Exp,
                                                 scale=SCALE)
                            nc.tensor.matmul(
                                av[0:97, :], v1[t][:], ex[:],
                                start=(t == 0), stop=(t == ST - 1))
                        nc.scalar.activation(
                            aoT[h][:, sblk * 512:(sblk + 1) * 512],
                            av[0:HD, :], AF.Copy)
                        nc.vector.tensor_copy(
                            sums[:, sblk * 512:(sblk + 1) * 512],
                            av[96:97, :])
                    nc.vector.reciprocal(sums[:], sums[:])
                    rsumb = tiny.tile([1, S], dt.bfloat16, tag="rsumb", bufs=1,
                                      name=f"rsumb_{l}_{h}")
                    nc.scalar.activation(rsumb[:], sums[:], AF.Copy)
                    rsb = bc.tile([HD, S], dt.bfloat16, tag="rsb", bufs=1,
                                  name=f"rsb_{l}_{h}")
                    nc.gpsimd.partition_broadcast(rsb[:], rsumb[:])
                    nc.vector.tensor_tensor(aoT[h][:], aoT[h][:], rsb[:],
                                            OP.mult)

                # ---- AllToAll attn out (bf16) ----
                ai = dram.tile([H, SR], dt.bfloat16, tag="a2a_in",
                               name=f"a2ai_{l}")
                for j in range(NCORE):
                    for h in range(HPC):
                        nc.sync.dma_start(
                            ai[j * DR + h * HD: j * DR + (h + 1) * HD, :],
                            aoT[h][:, j * SR:(j + 1) * SR])
                ao = dram.tile([H, SR], dt.bfloat16, tag="a2a_out",
                               name=f"a2ao_{l}")
                nc.gpsimd.collective_compute(
                    "AllToAll", OP.bypass, replica_groups=RG,
                    ins=[ai[:].opt()], outs=[ao[:].opt()])

                # ---- proj + residual (bf16 matmul, fp32 accumulate) ----
                prj = []
                for k in range(HT):
                    r = sb.tile([128, SR], dt.bfloat16, tag="prj", bufs=HT,
                                name=f"prj_{l}_{k}")
                    nc.sync.dma_start(r[:], ao[k * 128:(k + 1) * 128, :])
                    prj.append(r)
                for half in range(2):
                    paccs = [ps.tile([128, 512], dt.float32, tag="mm512",
                                     bufs=6, name=f"pps_{l}_{half}_{m5}")
                             for m5 in range(5)]
                    for k in range(HT):
                        w = wp.tile([128, 640], dt.bfloat16, tag="wfc2",
                                    bufs=4, name=f"wp_{l}_{half}_{k}")
                        nc.sync.dma_start(
                            w[:], inp[f"projwT_{l}"]
                            [k * 128:(k + 1) * 128,
                             half * 640:(half + 1) * 640])
                        for m5 in range(5):
                            nc.tensor.matmul(
                                paccs[m5][0:128, 0:SR],
                                w[:, m5 * 128:(m5 + 1) * 128], prj[k][:],
                                start=(k == 0), stop=(k == HT - 1))
                    for m5 in range(5):
                        m = half * 5 + m5
                        xs = xT[:, m * SR:(m + 1) * SR]
                        nc.vector.tensor_tensor(xs, xs,
                                                paccs[m5][0:128, 0:SR], OP.add)
                        nc.vector.tensor_scalar_add(
                            xs, xs, lw[f"projb_{l}"][:, m:m + 1])

                # ---- LN2 (local tokens only) ----
                ln2 = sb.tile([128, HT * SR], dt.bfloat16, tag="lnout", bufs=1,
                              name=f"ln2o_{l}")
                _layernorm(nc, pools, xT[:], lw[f"ln2w_{l}"][:],
                           lw[f"ln2b_{l}"][:], ln2[:], onesr, f"l2_{l}")

                # ---- fc1 + gelu: seq-local, full [H, I] weight streamed ----
                # out is inter-major: 40 tiles of [128, SR]
                gl = [sb.tile([128, SR], dt.bfloat16, tag="gl", bufs=40,
                              name=f"gl_{l}_{i}") for i in range(I // 128)]
                for mg in range(5):
                    w1s = []
                    for k in range(HT):
                        w = wp.tile([128, 1024], dt.bfloat16, tag="wfc1",
                                    bufs=HT, name=f"wf1_{l}_{mg}_{k}")
                        nc.sync.dma_start(
                            w[:], inp[f"fc1wT_{l}"]
                            [k * 128:(k + 1) * 128, mg * 1024:(mg + 1) * 1024])
                        w1s.append(w)
                    for m8 in range(8):
                        mi = mg * 8 + m8
                        acc = ps.tile([128, 512], dt.float32, tag="mm512",
                                      bufs=6, name=f"f1ps_{l}_{mi}")
                        for k in range(HT):
                            nc.tensor.matmul(
                                acc[0:128, 0:SR],
                                w1s[k][:, m8 * 128:(m8 + 1) * 128],
                                ln2[:, k * SR:(k + 1) * SR],
                                start=(k == 0), stop=(k == HT - 1))
                        nc.scalar.activation(
                            gl[mi][:], acc[0:128, 0:SR], AF.Gelu,
                            bias=lw[f"fc1b_{l}"][:, mi:mi + 1])

                # ---- fc2: seq-local, full [I, H] weight streamed ----
                for half in range(2):
                    accs = [ps.tile([128, 512], dt.float32, tag="mm512",
                                    bufs=6, name=f"f2ps_{l}_{half}_{m5}")
                            for m5 in range(5)]
                    for k2 in range(I // 128):
                        w = wp.tile([128, 640], dt.bfloat16, tag="wfc2",
                                    bufs=4, name=f"wf2_{l}_{half}_{k2}")
                        nc.sync.dma_start(
                            w[:], inp[f"fc2wT_{l}"]
                            [k2 * 128:(k2 + 1) * 128,
                             half * 640:(half + 1) * 640])
                        for m5 in range(5):
                            nc.tensor.matmul(
                                accs[m5][0:128, 0:SR],
                                w[:, m5 * 128:(m5 + 1) * 128], gl[k2][:],
                                start=(k2 == 0), stop=(k2 == I // 128 - 1))
                    for m5 in range(5):
                        m = half * 5 + m5
                        xs = xT[:, m * SR:(m + 1) * SR]
                        nc.vector.tensor_tensor(xs, xs,
                                                accs[m5][0:128, 0:SR], OP.add)
                        nc.vector.tensor_scalar_add(
                            xs, xs, lw[f"fc2b_{l}"][:, m:m + 1])

            # ================= merger =================
            MPART = int(os.environ.get("KERNEL_MERGER_PART", "9"))
            if MPART == 0:
                zt = sb.tile([128, GR], dt.float32, tag="mout", bufs=3)
                nc.vector.memset(zt[:], 0.0)
                for m in range(OT):
                    nc.sync.dma_start(out_r[m * 128:(m + 1) * 128, :], zt[:])
            else:
                mnw = tiny.tile([128, HT], dt.float32, tag="lw_ln1w", bufs=2)
                mnb = tiny.tile([128, HT], dt.float32, tag="lw_ln1b", bufs=2)
                nc.sync.dma_start(mnw[:], inp["mnw"])
                nc.sync.dma_start(mnb[:], inp["mnb"])
                mln = sb.tile([128, HT * SR], dt.float32, tag="mln", bufs=1)
                _layernorm(nc, pools, xT[:], mnw[:], mnb[:], mln[:], onesr, "mn")
                ysb = sb.tile([128, HT * SR], dt.bfloat16, tag="ln_xsq", bufs=1)
                for j in range(4):
                    nc.vector.tensor_copy(
                        ysb[:].rearrange("p (j t g) -> j p t g", j=4, t=HT)[j],
                        mln[:].rearrange("p (t g j) -> j p t g", t=HT, j=4)[j])
                yagi = dram.tile([MH, GR], dt.bfloat16, tag="y_agi")
                nc.sync.dma_start(
                    yagi[:].rearrange("(j t p) g -> p (j t) g", j=4, t=HT),
                    ysb[:].rearrange("p (j t g) -> p (j t) g", j=4, t=HT))
                yago = dram.tile([NCORE * MH, GR], dt.bfloat16, addr_space="Shared",
                                 tag="y_ago")
                nc.gpsimd.collective_compute(
                    "AllGather", OP.bypass, replica_groups=RG,
                    ins=[yagi[:].opt()], outs=[yago[:].opt()])

                mfc1b = tiny.tile([128, IT], dt.float32, tag="lw_fc1b", bufs=2)
                nc.sync.dma_start(mfc1b[:], inp["mfc1b"])
                mg = [sb.tile([128, NCORE * GR], dt.bfloat16, tag=f"gT{i}", bufs=1,
                              name=f"mg{i}") for i in range(IT)]
                yagov = yago[:].rearrange("(r k p) g -> k p r g", r=NCORE, k=MT)
                maccs = [ps.tile([128, 512], dt.float32, tag="mm512", bufs=6,
                                 name=f"m1ps_{m}") for m in range(IT)]
                for k in range(MT):
                    w = wp.tile([128, IR], dt.bfloat16, tag="mw1", bufs=2,
                                name=f"mw1_{k}")
                    nc.sync.dma_start(
                        w[:], inp["mfc1wT"][k * 128:(k + 1) * 128, :])
                    r = sb.tile([128, 512], dt.bfloat16, tag="yrhs", bufs=2,
                                name=f"mrhs_{k}")
                    nc.sync.dma_start(r[:].rearrange("p (r g) -> p r g", r=NCORE),
                                      yagov[k])
                    for m in range(IT):
                        nc.tensor.matmul(maccs[m][:],
                                         w[:, m * 128:(m + 1) * 128], r[:],
                                         start=(k == 0), stop=(k == MT - 1))
                for m in range(IT):
                    nc.scalar.activation(mg[m][:], maccs[m][:], AF.Gelu,
                                         bias=mfc1b[:, m:m + 1])

                mrsi = dram.tile([NCORE * OUT, GR], dt.bfloat16, tag="mrs_in")
                mrsiv = mrsi[:].rearrange("(j t p) g -> j t p g", j=NCORE, t=OT)
                for m in range(OT):
                    acc = ps.tile([128, 512], dt.float32, tag="mm512", bufs=6,
                                  name=f"m2ps_{m}")
                    for k in range(IT):
                        w = wp.tile([128, 128], dt.bfloat16, tag="mw2", bufs=2,
                                    name=f"mw2_{m}_{k}")
                        nc.sync.dma_start(
                            w[:], inp["mfc2wT"]
                            [k * 128:(k + 1) * 128, m * 128:(m + 1) * 128])
                        nc.tensor.matmul(acc[:], w[:], mg[k][:],
                                         start=(k == 0), stop=(k == IT - 1))
                    ev = sb.tile([128, 512], dt.bfloat16, tag="f2ev", bufs=2,
                                 name=f"m2ev_{m}")
                    nc.scalar.activation(ev[:], acc[:], AF.Copy)
                    nc.sync.dma_start(
                        mrsiv[:, m].transpose([1, 0, 2]),
                        ev[:].rearrange("p (j g) -> p j g", j=NCORE))
                mrso = dram.tile([OUT, GR], dt.bfloat16, tag="mrs_out")
                nc.gpsimd.collective_compute(
                    "ReduceScatter", OP.add, replica_groups=RG,
                    ins=[mrsi[:].opt()], outs=[mrso[:].opt()])
                mfc2b = tiny.tile([128, OT], dt.float32, tag="lw_m2b", bufs=1)
                nc.sync.dma_start(mfc2b[:], inp["mfc2b"])
                for m in range(OT):
                    r = sb.tile([128, GR], dt.bfloat16, tag="rsout", bufs=3,
                                name=f"mro_{m}")
                    nc.sync.dma_start(r[:], mrso[m * 128:(m + 1) * 128, :])
                    o = sb.tile([128, GR], dt.float32, tag="mout", bufs=3,
                                name=f"mo_{m}")
                    nc.scalar.activation(o[:], r[:], AF.Identity,
                                         bias=mfc2b[:, m:m + 1])
                    nc.sync.dma_start(out_r[m * 128:(m + 1) * 128, :], o[:])

    nc.compile()
    return nc


def _prep_inputs(inputs, depth=DEPTH):
    f32 = lambda a: np.ascontiguousarray(np.asarray(a), dtype=np.float32)
    bf = lambda a: np.ascontiguousarray(np.asarray(a, dtype=np.float32)
                                        .astype(BF))
    f8 = lambda a: np.ascontiguousarray(np.asarray(a, dtype=np.float32)
                                        .astype(ml_dtypes.float8_e4m3))
    hs = f32(inputs["hidden_states"])
    cos, sin = f32(inputs["cos"]), f32(inputs["sin"])
    sgnsin = np.concatenate([-sin[:, :HD // 2], sin[:, HD // 2:]], axis=1)
    qkv_w, qkv_b = f32(inputs["qkv_w"]), f32(inputs["qkv_b"])
    proj_w = f32(inputs["proj_w"])
    fc1_w, fc2_w = f32(inputs["fc1_w"]), f32(inputs["fc2_w"])

    def fold(vec, cols):
        return np.ascontiguousarray(f32(vec).reshape(cols, -1).T)

    in_maps = []
    for c in range(NCORE):
        m = {
            "x0": np.ascontiguousarray(hs[c * SR:(c + 1) * SR].T),
            "cosT": bf(cos.T),
            "sgnsinT": bf(sgnsin.T),
            "ones_col": np.ones((128, 1), np.float32),
        }
        h0, h1 = HPC * c, HPC * c + 1
        for l in range(depth):
            rows, bias = [], []
            for part in range(3):  # q, k, v
                for h in (h0, h1):
                    rows.append(qkv_w[l][part * H + h * HD:
                                         part * H + (h + 1) * HD, :])
                    bias.append(qkv_b[l][part * H + h * HD:
                                         part * H + (h + 1) * HD])
            m[f"qkvwT_{l}"] = bf(np.concatenate(rows, axis=0).T)
            m[f"qkvb_{l}"] = np.ascontiguousarray(np.stack(bias, axis=1))
            m[f"projwT_{l}"] = bf(proj_w[l].T)
            m[f"projb_{l}"] = fold(inputs["proj_b"][l], HT)
            m[f"ln1w_{l}"] = fold(inputs["ln1_w"][l], HT)
            m[f"ln1b_{l}"] = fold(inputs["ln1_b"][l], HT)
            m[f"ln2w_{l}"] = fold(inputs["ln2_w"][l], HT)
            m[f"ln2b_{l}"] = fold(inputs["ln2_b"][l], HT)
            m[f"fc1wT_{l}"] = bf(fc1_w[l].T)
            m[f"fc1b_{l}"] = fold(np.asarray(inputs["fc1_b"])[l], I // 128)
            m[f"fc2wT_{l}"] = bf(fc2_w[l].T)
            m[f"fc2b_{l}"] = fold(inputs["fc2_b"][l], HT)
        m["mnw"] = fold(inputs["mnorm_w"], HT)
        m["mnb"] = fold(inputs["mnorm_b"], HT)
        m["mfc1wT"] = bf(f32(inputs["mfc1_w"])[c * IR:(c + 1) * IR, :].T)
        m["mfc1b"] = fold(f32(inputs["mfc1_b"])[c * IR:(c + 1) * IR], IT)
        m["mfc2wT"] = bf(f32(inputs["mfc2_w"])[:, c * IR:(c + 1) * IR].T)
        m["mfc2b"] = fold(inputs["mfc2_b"], OT)
        in_maps.append(m)
    return in_maps


def kernel(**inputs) -> np.ndarray:
    if "nc" not in _cached:
        _cached["nc"] = build_nc()
    nc = _cached["nc"]
    in_maps = _prep_inputs(inputs)
    res = run_bass_kernel_spmd(
        nc, in_maps, core_ids=list(range(NCORE)),
        trace=bool(int(os.environ.get("KERNEL_TRACE", "0"))))
    _cached["last_result"] = res
    out = np.empty((S // 4, OUT), np.float32)
    for c in range(NCORE):
        out[c * GR:(c + 1) * GR, :] = res.results[c]["out_r"].T
    return out

